# revision 8
# baseline (speedup 1.0000x reference)
"""Multi-head causal attention (b=2, T=2048, d=1024, 16 heads) on 8 TRN2 cores.

Sharding: tensor-parallel over heads, 2 heads per core, both batch elements on
every core.  Per core:
  - QKV projections (contraction over d_in=1024) with x^T resident in SBUF;
    Q^T/K^T land in [channel, token] layout, V in [token, channel] layout
    augmented with a ones column (softmax denominator).
  - Attention per (head, block) unit in transposed-score layout S^T[kpos, q],
    where block = (batch, 512-token q range), processed block-major with both
    heads per block: scores (diagonal tiles trimmed to the valid q range) ->
    exp (max-free softmax, scores bounded) -> causal mask on the diagonal
    128x128 tiles -> attn@V with the exp'd scores stationary, producing
    ctx[q, ch]; denominator from the ones column -> reciprocal + broadcast
    multiply.  ctx is transposed back to ctx^T[ch, q] with PE transposes and
    staged per block.
  - Two token-split AllToAlls re-shard ctx from head-sharded to token-sharded:
    each slot carries BOTH local heads for a 64-token slice per block.  A2A#1
    covers blocks 0-5 and overlaps the tail of attention; A2A#2 covers only
    blocks 6-7 (262KB), minimizing the exposed collective latency.
  - Received ctx channels land in natural Wo row order (core-major), so the
    out-projection contracts full 128-row subtiles: token pairs (2 blocks =
    128 tokens) x 8 subtiles x 2 column halves.  Pairs 0-2 run under the
    A2A#2 window; pair 3 lands right after A2A#2.  Junk matmuls (reading
    resident q_sb) keep the PE p-state ramped across the collective windows.
  - QKV projection work is dripped between attention score groups in program
    order so the in-order PE queue always has independent matmuls while the
    Activation engine works through the exps.
Host side only shards/casts inputs and concatenates the 8 output slices.
"""

import sys

sys.path.insert(0, "/opt/trn_rl_repo")

import numpy as np
import ml_dtypes

import concourse.bass as bass
import concourse.mybir as mybir
import concourse.tile as tile
from concourse.tile import add_dep_helper
from concourse import bacc
from concourse.bass_utils import run_bass_kernel_spmd

B = 2
T = 2048
D = 1024
DH = 64
HL = 2  # heads per core
P = 128
CI = D // P  # 8 contraction subtiles
TQ = B * T  # 4096
QB = 512  # q block
NB = TQ // QB  # 8 blocks total (batch-major)
NKT = T // P  # 16 kpos tiles per batch
NW = 8  # a2a slots == cores
SPLIT = 4  # blocks 0..SPLIT-1 in A2A#1, rest in A2A#2
SL = QB // NW  # 64 tokens per (block, slot) slice
F32 = mybir.dt.float32
BF16 = mybir.dt.bfloat16
EXP = mybir.ActivationFunctionType.Exp

JUNK_PRE = 0    # 512-row junk matmuls before D1 (cf1 lands during attention)
JUNK_MID = 52   # 512-row junk matmuls between D1 and D2 (bridge to cf2)

_CACHE = {}


def _build():
    nc = bacc.Bacc("TRN2", target_bir_lowering=False, num_devices=8)
    xt = nc.dram_tensor("xt", [D, TQ], BF16, kind="ExternalInput")
    wq = nc.dram_tensor("wq", [D, P], BF16, kind="ExternalInput")
    wk = nc.dram_tensor("wk", [D, P], BF16, kind="ExternalInput")
    wv = nc.dram_tensor("wv", [D, P], BF16, kind="ExternalInput")
    wo = nc.dram_tensor("wo", [D, D], BF16, kind="ExternalInput")
    bob = nc.dram_tensor("bob", [P, D], F32, kind="ExternalInput")
    mskd = nc.dram_tensor("mskd", [P, P], BF16, kind="ExternalInput")
    iden = nc.dram_tensor("iden", [P, P], BF16, kind="ExternalInput")
    out = nc.dram_tensor("out", [QB, D], F32, kind="ExternalOutput")

    xt_r = xt.rearrange("(s p) t -> p s t", p=P)

    # x chunk schedule: small first chunks so the first QK matmuls start early
    CHUNKS = [(0, 128), (128, 384)] + [(512 * k, 512) for k in range(1, NB)]

    with tile.TileContext(nc) as tc:
        with (
            tc.tile_pool(name="const", bufs=1) as const,
            tc.tile_pool(name="dram", bufs=1, space="DRAM") as dram,
        ):
            xt_sb = const.tile([P, CI, TQ], BF16)
            wq_sb = const.tile([P, CI, P], BF16)
            wk_sb = const.tile([P, CI, P], BF16)
            wv_sb = const.tile([P, CI, P], BF16)
            wo_sb = const.tile([P, CI, D], BF16)
            bob_sb = const.tile([P, D], F32)
            mskd_sb = const.tile([P, P], BF16)
            iden_sb = const.tile([P, P], BF16)
            q_sb = const.tile([P, TQ], BF16)
            k_sb = const.tile([P, TQ], BF16)
            # V in [token, channel] layout + ones column: [kpos_tile, head, 65]
            v_sb = const.tile([P, B * NKT, HL, DH + 1], BF16)
            # exp'd scores for the current (head, block): all kpos tiles,
            # triple-buffered per unit
            at_sb = const.tile([P, 3, NKT, QB], BF16)
            # ctx^T staging: head h on partitions 64h..64h+64, per block
            ctxT_sb = const.tile([P, NB, QB], BF16)
            # re-sharded full-channel ctx for my token slices:
            # [ch-in-subtile, subtile(=src core), blk*64+t]
            cf_sb = const.tile([P, CI, QB], BF16)

            # wq + first x chunks unblock the first Q projection
            nc.sync.dma_start(wq_sb[:], wq.rearrange("(s p) m -> p s m", p=P))
            prev_dma = nc.sync.dma_start(xt_sb[:, :, 0:128], xt_r[:, :, 0:128])
            nc.sync.dma_start(wk_sb[:], wk.rearrange("(s p) m -> p s m", p=P))
            d = nc.sync.dma_start(xt_sb[:, :, 128:512], xt_r[:, :, 128:512])
            add_dep_helper(d.ins, prev_dma.ins, sync=True, reason="xt order")
            prev_dma = d
            nc.sync.dma_start(wv_sb[:], wv.rearrange("(s p) m -> p s m", p=P))
            nc.sync.dma_start(mskd_sb[:], mskd[:])
            nc.sync.dma_start(iden_sb[:], iden[:])
            # remaining x^T chunks, chained so chunk k arrives in order
            for k in range(1, NB):
                d = nc.sync.dma_start(
                    xt_sb[:, :, k * QB : (k + 1) * QB],
                    xt_r[:, :, k * QB : (k + 1) * QB],
                )
                add_dep_helper(d.ins, prev_dma.ins, sync=True, reason="xt order")
                prev_dma = d
            # weights needed only by the output projection come last
            d = nc.sync.dma_start(wo_sb[:], wo.rearrange("(s p) m -> p s m", p=P))
            add_dep_helper(d.ins, prev_dma.ins, sync=True, reason="wo after xt")
            d = nc.sync.dma_start(bob_sb[:], bob[:])
            add_dep_helper(d.ins, prev_dma.ins, sync=True, reason="bob after xt")
            nc.vector.memset(v_sb[:, :, :, DH : DH + 1], 1.0)

            # token-split A2A buffers: slot j = [128 ch, blocks, 64 tok]
            a2a_in1 = dram.tile([NW, P, SPLIT, SL], BF16, name="a2a_in1")
            a2a_out1 = dram.tile([NW, P, SPLIT, SL], BF16, name="a2a_out1")
            a2a_in2 = dram.tile([NW, P, NB - SPLIT, SL], BF16, name="a2a_in2")
            a2a_out2 = dram.tile([NW, P, NB - SPLIT, SL], BF16, name="a2a_out2")

            with (
                tc.tile_pool(name="psC", bufs=1, space="PSUM") as psC,
                tc.tile_pool(name="psT", bufs=1, space="PSUM") as psT,
                tc.tile_pool(name="sbm", bufs=2) as sbm,
            ):
                # ---- QKV step emitters (one PSUM tile each) ----

                def emit_qk_step(t0, tlen, dst, w):
                    pt = psA.tile([P, QB], F32, tag="qk", name="pt")
                    for s in range(CI):
                        nc.tensor.matmul(
                            pt[:, 0:tlen],
                            w[:, s, :],
                            xt_sb[:, s, t0 : t0 + tlen],
                            start=(s == 0),
                            stop=(s == CI - 1),
                        )
                    nc.vector.tensor_copy(dst[:, t0 : t0 + tlen], pt[:, 0:tlen])

                def emit_v_step(t0, tlen):
                    tt0 = t0 // P
                    ntt = tlen // P
                    pv = psA.tile([P, QB], F32, tag="qk", name="pv")
                    for tt in range(ntt):
                        for s in range(CI):
                            nc.tensor.matmul(
                                pv[:, tt * P : (tt + 1) * P],
                                xt_sb[:, s, (tt0 + tt) * P : (tt0 + tt + 1) * P],
                                wv_sb[:, s, :],
                                start=(s == 0),
                                stop=(s == CI - 1),
                            )
                    nc.vector.tensor_copy(
                        v_sb[:, tt0 : tt0 + ntt, :, 0:DH],
                        pv[:, 0:tlen].rearrange("p (t h d) -> p t h d", t=ntt, h=HL),
                    )

                def qkv_steps():
                    for t0, tlen in CHUNKS:
                        fi = t0 // QB
                        yield (fi, lambda t0=t0, tlen=tlen: emit_qk_step(t0, tlen, q_sb, wq_sb))
                        yield (fi, lambda t0=t0, tlen=tlen: emit_qk_step(t0, tlen, k_sb, wk_sb))
                        yield (fi, lambda t0=t0, tlen=tlen: emit_v_step(t0, tlen))

                qkv_iter = qkv_steps()
                qkv_pending = []  # one lookahead slot
                drip_tick = [0]
                ep_queue = []  # deferred per-unit epilogue thunks

                def qkv_drip(max_steps):
                    n = 0
                    while n < max_steps:
                        if qkv_pending:
                            _, thunk = qkv_pending.pop(0)
                            thunk()
                            n += 1
                            continue
                        nxt = next(qkv_iter, None)
                        if nxt is None:
                            return
                        qkv_pending.append(nxt)

                def qkv_flush(through_blk):
                    while True:
                        if qkv_pending:
                            fi, thunk = qkv_pending[0]
                            if fi > through_blk:
                                return
                            qkv_pending.pop(0)
                            thunk()
                            continue
                        nxt = next(qkv_iter, None)
                        if nxt is None:
                            return
                        qkv_pending.append(nxt)

                def ep_drip(n=1):
                    for _ in range(n):
                        if ep_queue:
                            ep_queue.pop(0)()

                def ep_flush():
                    while ep_queue:
                        ep_queue.pop(0)()

                # ---- attention emitter ----

                def emit_attn(ui, h, blk, pools, drip=False):
                    """Score groups for unit (head h, block blk).  Returns a
                    thunk emitting the unit's attn@V + epilogue — invoked
                    after the NEXT unit's score groups so the Activation
                    engine's exp stream never waits on the PE draining attn@V
                    at a unit boundary."""
                    b, qb = blk // 4, blk % 4
                    at = at_sb[:, ui % 3]
                    hp = DH * h
                    tb = b * T
                    kb = b * NKT
                    qs0 = tb + qb * QB
                    nkt = 4 * (qb + 1)  # kpos tiles up to the diagonal
                    groups = []
                    kt = 0
                    gi = 0
                    while kt < nkt:
                        pool, cap = pools[gi % len(pools)]
                        n = min(cap, nkt - kt)
                        groups.append((kt, n, pool, cap))
                        kt += n
                        gi += 1

                    for g, (kt0, n, pool, cap) in enumerate(groups):
                        sps = pool.tile([P, cap, QB], F32, tag="s", name="sps")
                        # diagonal tiles are exp'd in pairs; trim each tile's
                        # matmul only to its PAIR's q start so the paired exp
                        # never reads unwritten PSUM
                        i0 = max(0, 4 * qb - kt0)  # first diagonal index
                        for i in range(n):
                            ktg = kt0 + i
                            dq = ktg - 4 * qb
                            if dq >= 0:
                                dq_pair = (kt0 + i0 + 2 * ((i - i0) // 2)) - 4 * qb
                                lo = dq_pair * P
                            else:
                                lo = 0
                            nc.tensor.matmul(
                                sps[:, i, lo:QB],
                                k_sb[hp : hp + DH, tb + ktg * P : tb + (ktg + 1) * P],
                                q_sb[hp : hp + DH, qs0 + lo : qs0 + QB],
                                start=True,
                                stop=True,
                                tile_position=(hp, 0),
                            )
                        # exp: off-diagonal tiles full-width in one run,
                        # diagonal tiles in pairs trimmed to the pair's start
                        i = 0
                        while i < n:
                            dq0 = (kt0 + i) - 4 * qb
                            if dq0 < 0:
                                j = i
                                while j < n and (kt0 + j) - 4 * qb < 0:
                                    j += 1
                                nc.scalar.activation(
                                    at[:, kt0 + i : kt0 + j, :],
                                    sps[:, i:j, :],
                                    EXP,
                                    scale=0.125,
                                )
                                i = j
                            else:
                                j = min(i + 2, n)
                                lo = dq0 * P
                                nc.scalar.activation(
                                    at[:, kt0 + i : kt0 + j, lo:QB],
                                    sps[:, i:j, lo:QB],
                                    EXP,
                                    scale=0.125,
                                )
                                i = j
                        # causal mask on the diagonal tiles
                        for i in range(n):
                            dq = (kt0 + i) - 4 * qb
                            if dq >= 0:
                                a = at[:, kt0 + i, dq * P : (dq + 1) * P]
                                nc.vector.tensor_tensor(
                                    a, a, mskd_sb[:], mybir.AluOpType.mult
                                )
                        ep_drip(2)
                        if drip:
                            drip_tick[0] = (drip_tick[0] + 1) % 2
                            if drip_tick[0] == 0:
                                qkv_drip(1)

                    def av_ep():
                        ep_flush()
                        cps = psC.tile([P, 4, DH + 1], F32, tag="ctx", name="cps")
                        for qs in range(4):
                            qg = 4 * qb + qs
                            for kt in range(qg + 1):
                                nc.tensor.matmul(
                                    cps[:, qs, :],
                                    at[:, kt, qs * P : (qs + 1) * P],
                                    v_sb[:, kb + kt, h, :],
                                    start=(kt == 0),
                                    stop=(kt == qg),
                                )
                        ctxn = sbm.tile([P, 4, DH], BF16, tag="ctxn", name="ctxn")
                        den_sb = sbm.tile([P, 4], F32, tag="den", name="den_sb")
                        nc.vector.reciprocal(den_sb[:], cps[:, :, DH])
                        tp_box = []

                        def ep_qs(qs):
                            den = den_sb[:, qs : qs + 1]
                            denb = bass.AP(
                                tensor=den.tensor,
                                offset=den.offset,
                                ap=[list(den.ap[0]), [0, DH]],
                            )
                            nc.vector.tensor_tensor(
                                ctxn[:, qs, :],
                                cps[:, qs, 0:DH],
                                denb,
                                mybir.AluOpType.mult,
                            )
                            if not tp_box:
                                tp_box.append(
                                    psT.tile([DH, 4, P], BF16, tag="tp", name="tp")
                                )
                            nc.tensor.transpose(
                                tp_box[0][:, qs, :], ctxn[:, qs, :], iden_sb[:]
                            )

                        def ep_stage():
                            nc.vector.tensor_copy(
                                ctxT_sb[hp : hp + DH, blk, :],
                                tp_box[0][:].rearrange("d a p -> d (a p)"),
                            )
                            src = ctxT_sb[hp : hp + DH, blk, :].rearrange(
                                "d (j t) -> j d t", j=NW
                            )
                            if blk < SPLIT:
                                nc.sync.dma_start(
                                    a2a_in1[:, hp : hp + DH, blk, :], src
                                )
                            else:
                                nc.sync.dma_start(
                                    a2a_in2[:, hp : hp + DH, blk - SPLIT, :], src
                                )

                        for qs in range(4):
                            ep_queue.append(lambda qs=qs: ep_qs(qs))
                        ep_queue.append(ep_stage)

                    return av_ep

                def emit_a2a(i):
                    a_in = a2a_in1 if i == 0 else a2a_in2
                    a_out = a2a_out1 if i == 0 else a2a_out2
                    nblk = SPLIT if i == 0 else NB - SPLIT
                    c0 = 0 if i == 0 else SPLIT * SL
                    nc.gpsimd.collective_compute(
                        "AllToAll",
                        mybir.AluOpType.bypass,
                        replica_groups=[[0, 1, 2, 3, 4, 5, 6, 7]],
                        ins=[a_in.opt()],
                        outs=[a_out.opt()],
                    )
                    # land in two halves (src cores 0-3, 4-7) for pipelining
                    for g2 in range(2):
                        nc.sync.dma_start(
                            cf_sb[:, 4 * g2 : 4 * (g2 + 1), c0 : c0 + nblk * SL],
                            a_out[4 * g2 : 4 * (g2 + 1)].rearrange(
                                "s c b t -> c s (b t)"
                            ),
                        )

                # ---- attention: block-major, both heads per block ----
                units = [(h, blk) for blk in range(NB) for h in range(HL)]
                units_A = [u for u in units if u[1] < 6]
                units_B = [u for u in units if u[1] >= 6]
                trig_a2a1 = (HL - 1, SPLIT - 1)
                av_prev = None
                av_prev_unit = None
                ucount = 0

                def pump(av_new, unit):
                    nonlocal av_prev, av_prev_unit, ucount
                    if av_prev is not None:
                        av_prev()
                        if av_prev_unit == trig_a2a1:
                            ep_flush()
                            emit_a2a(0)
                    av_prev = av_new
                    av_prev_unit = unit
                    ucount += 1

                with (
                    tc.tile_pool(name="psA", bufs=2, space="PSUM") as psA,
                    tc.tile_pool(name="psS", bufs=1, space="PSUM") as psS,
                    tc.tile_pool(name="psSb", bufs=1, space="PSUM") as psSb,
                ):
                    pools1 = [(psS, 2), (psSb, 2)]
                    for h, blk in units_A:
                        qkv_flush(blk)  # deps for this unit
                        av_new = emit_attn(ucount, h, blk, pools1, drip=True)
                        pump(av_new, (h, blk))
                    qkv_flush(NB)
                with (
                    tc.tile_pool(name="psS4", bufs=1, space="PSUM") as psS4,
                    tc.tile_pool(name="psS5", bufs=1, space="PSUM") as psS5,
                ):
                    pools2 = [(psS4, 3), (psS5, 3)]
                    for h, blk in units_B:
                        av_new = emit_attn(ucount, h, blk, pools2)
                        pump(av_new, (h, blk))
                    av_prev()
                    ep_flush()
                emit_a2a(1)

            # ---- phase 3: output projection, token pairs of 128 ----
            with (
                tc.tile_pool(name="psO", bufs=2, space="PSUM") as psO,
                tc.tile_pool(name="psJ", bufs=1, space="PSUM") as psJ,
                tc.tile_pool(name="osb", bufs=2) as osb,
            ):
                jt = psJ.tile([P, QB], F32, tag="junk", name="jt")

                def emit_junk(n):
                    for _ in range(n):
                        nc.tensor.matmul(
                            jt[:],
                            q_sb[0:DH, 0:P],
                            q_sb[0:DH, 0:QB],
                            start=True,
                            stop=True,
                            tile_position=(0, 0),
                            skip_group_check=True,
                        )

                def emit_pair(p):
                    # n2-outer: the first column half's bias+store overlaps
                    # the second half's matmuls
                    ot = osb.tile([P, D], F32, tag="o", name="ot")
                    for n2 in range(2):
                        po = psO.tile([P, 512], F32, tag=f"po{n2}", name="po")
                        for s in range(CI):
                            nc.tensor.matmul(
                                po[:],
                                cf_sb[:, s, p * P : (p + 1) * P],
                                wo_sb[:, s, n2 * 512 : (n2 + 1) * 512],
                                start=(s == 0),
                                stop=(s == CI - 1),
                            )
                        nc.vector.tensor_tensor(
                            ot[:, n2 * 512 : (n2 + 1) * 512],
                            po[:],
                            bob_sb[:, n2 * 512 : (n2 + 1) * 512],
                            mybir.AluOpType.add,
                        )
                        nc.sync.dma_start(
                            out[p * P : (p + 1) * P, n2 * 512 : (n2 + 1) * 512],
                            ot[:, n2 * 512 : (n2 + 1) * 512],
                        )

                emit_junk(JUNK_PRE)
                for p in range(SPLIT // 2):
                    emit_pair(p)
                emit_junk(JUNK_MID)
                for p in range(SPLIT // 2, 4):
                    emit_pair(p)
    nc.finalize()
    return nc


def _get_nc():
    if "nc" not in _CACHE:
        _CACHE["nc"] = _build()
    return _CACHE["nc"]


def kernel(x, Wq, Wk, Wv, Wo, bo, **run_kwargs):
    x = np.asarray(x, np.float32)
    Wq = np.asarray(Wq, np.float32)
    Wk = np.asarray(Wk, np.float32)
    Wv = np.asarray(Wv, np.float32)
    Wo = np.asarray(Wo, np.float32)
    bo = np.asarray(bo, np.float32)

    xt16 = np.ascontiguousarray(x.reshape(TQ, D).T).astype(ml_dtypes.bfloat16)
    wo16 = Wo.astype(ml_dtypes.bfloat16)
    bob = np.ascontiguousarray(np.broadcast_to(bo, (P, D))).astype(np.float32)
    ii = np.arange(P)[:, None]
    jj = np.arange(P)[None, :]
    mskd = (jj >= ii).astype(ml_dtypes.bfloat16)
    iden = np.eye(P, dtype=ml_dtypes.bfloat16)

    in_maps = []
    for c in range(8):
        sl = slice(P * c, P * (c + 1))
        in_maps.append(
            {
                "xt": xt16,
                "wq": np.ascontiguousarray(Wq[:, sl]).astype(ml_dtypes.bfloat16),
                "wk": np.ascontiguousarray(Wk[:, sl]).astype(ml_dtypes.bfloat16),
                "wv": np.ascontiguousarray(Wv[:, sl]).astype(ml_dtypes.bfloat16),
                "wo": wo16,
                "bob": bob,
                "mskd": mskd,
                "iden": iden,
            }
        )

    nc = _get_nc()
    res = run_bass_kernel_spmd(nc, in_maps, core_ids=list(range(8)), **run_kwargs)

    outp = np.empty((B, T, D), np.float32)
    for c in range(8):
        arr = res.results[c]["out"].reshape(NB, SL, D)
        for blk in range(NB):
            b, qb = blk // 4, blk % 4
            t0 = qb * QB + c * SL
            outp[b, t0 : t0 + SL, :] = arr[blk]
    return outp


# revision 12
# speedup vs baseline: 1.0158x; 1.0158x over previous
"""Multi-head causal attention (b=2, T=2048, d=1024, 16 heads) on 8 TRN2 cores.

Sharding: tensor-parallel over heads, 2 heads per core, both batch elements on
every core.  Per core:
  - QKV projections (contraction over d_in=1024) with x^T resident in SBUF;
    Q^T/K^T land in [channel, token] layout, V in [token, channel] layout
    augmented with a ones column (softmax denominator).
  - Attention per (head, block) unit in transposed-score layout S^T[kpos, q],
    where block = (batch, 512-token q range), processed block-major with both
    heads per block: scores (diagonal tiles trimmed to the valid q range) ->
    exp (max-free softmax, scores bounded) -> causal mask on the diagonal
    128x128 tiles -> attn@V with the exp'd scores stationary, producing
    ctx[q, ch]; denominator from the ones column -> reciprocal + broadcast
    multiply.  ctx is transposed back to ctx^T[ch, q] with PE transposes and
    staged per block.
  - Two token-split AllToAlls re-shard ctx from head-sharded to token-sharded:
    each slot carries BOTH local heads for a 64-token slice per block.  A2A#1
    covers blocks 0-5 and overlaps the tail of attention; A2A#2 covers only
    blocks 6-7 (262KB), minimizing the exposed collective latency.
  - Received ctx channels land in natural Wo row order (core-major), so the
    out-projection contracts full 128-row subtiles: token pairs (2 blocks =
    128 tokens) x 8 subtiles x 2 column halves.  Pairs 0-2 run under the
    A2A#2 window; pair 3 lands right after A2A#2.  Junk matmuls (reading
    resident q_sb) keep the PE p-state ramped across the collective windows.
  - QKV projection work is dripped between attention score groups in program
    order so the in-order PE queue always has independent matmuls while the
    Activation engine works through the exps.
Host side only shards/casts inputs and concatenates the 8 output slices.
"""

import sys

sys.path.insert(0, "/opt/trn_rl_repo")

import numpy as np
import ml_dtypes

import concourse.bass as bass
import concourse.mybir as mybir
import concourse.tile as tile
from concourse.tile import add_dep_helper
from concourse import bacc
from concourse.bass_utils import run_bass_kernel_spmd

B = 2
T = 2048
D = 1024
DH = 64
HL = 2  # heads per core
P = 128
CI = D // P  # 8 contraction subtiles
TQ = B * T  # 4096
QB = 512  # q block
NB = TQ // QB  # 8 blocks total (batch-major)
NKT = T // P  # 16 kpos tiles per batch
NW = 8  # a2a slots == cores
SPLIT = 4  # blocks 0..SPLIT-1 in A2A#1, rest in A2A#2
SL = QB // NW  # 64 tokens per (block, slot) slice
F32 = mybir.dt.float32
BF16 = mybir.dt.bfloat16
EXP = mybir.ActivationFunctionType.Exp

JUNK_PRE = 0    # 512-row junk matmuls before D1 (cf1 lands during attention)
JUNK_MID = 100  # 512-row junk matmuls between D1 and D2 (bridge to cf2)

_CACHE = {}


def _build():
    nc = bacc.Bacc("TRN2", target_bir_lowering=False, num_devices=8)
    xt = nc.dram_tensor("xt", [D, TQ], BF16, kind="ExternalInput")
    wq = nc.dram_tensor("wq", [D, P], BF16, kind="ExternalInput")
    wk = nc.dram_tensor("wk", [D, P], BF16, kind="ExternalInput")
    wv = nc.dram_tensor("wv", [D, P], BF16, kind="ExternalInput")
    wo = nc.dram_tensor("wo", [D, D], BF16, kind="ExternalInput")
    bob = nc.dram_tensor("bob", [P, D], F32, kind="ExternalInput")
    mskd = nc.dram_tensor("mskd", [P, P], BF16, kind="ExternalInput")
    iden = nc.dram_tensor("iden", [P, P], BF16, kind="ExternalInput")
    out = nc.dram_tensor("out", [QB, D], F32, kind="ExternalOutput")

    xt_r = xt.rearrange("(s p) t -> p s t", p=P)

    # x chunk schedule: small first chunks so the first QK matmuls start early
    CHUNKS = [(0, 128), (128, 384)] + [(512 * k, 512) for k in range(1, NB)]

    with tile.TileContext(nc) as tc:
        with (
            tc.tile_pool(name="const", bufs=1) as const,
            tc.tile_pool(name="dram", bufs=1, space="DRAM") as dram,
        ):
            xt_sb = const.tile([P, CI, TQ], BF16)
            wq_sb = const.tile([P, CI, P], BF16)
            wk_sb = const.tile([P, CI, P], BF16)
            wv_sb = const.tile([P, CI, P], BF16)
            wo_sb = const.tile([P, CI, D], BF16)
            bob_sb = const.tile([P, D], F32)
            mskd_sb = const.tile([P, P], BF16)
            iden_sb = const.tile([P, P], BF16)
            q_sb = const.tile([P, TQ], BF16)
            k_sb = const.tile([P, TQ], BF16)
            # V in [token, channel] layout + ones column: [kpos_tile, head, 65]
            v_sb = const.tile([P, B * NKT, HL, DH + 1], BF16)
            # exp'd scores for the current (head, block): all kpos tiles,
            # triple-buffered per unit
            at_sb = const.tile([P, 3, NKT, QB], BF16)
            # ctx^T staging: head h on partitions 64h..64h+64, per block
            ctxT_sb = const.tile([P, NB, QB], BF16)
            # re-sharded full-channel ctx for my token slices:
            # [ch-in-subtile, subtile(=src core), blk*64+t]
            cf_sb = const.tile([P, CI, QB], BF16)

            # wq + first x chunks unblock the first Q projection
            nc.sync.dma_start(wq_sb[:], wq.rearrange("(s p) m -> p s m", p=P))
            prev_dma = nc.sync.dma_start(xt_sb[:, :, 0:128], xt_r[:, :, 0:128])
            nc.sync.dma_start(wk_sb[:], wk.rearrange("(s p) m -> p s m", p=P))
            nc.sync.dma_start(wv_sb[:], wv.rearrange("(s p) m -> p s m", p=P))
            d = nc.sync.dma_start(xt_sb[:, :, 128:512], xt_r[:, :, 128:512])
            add_dep_helper(d.ins, prev_dma.ins, sync=True, reason="xt order")
            prev_dma = d
            nc.sync.dma_start(mskd_sb[:], mskd[:])
            nc.sync.dma_start(iden_sb[:], iden[:])
            # remaining x^T chunks, chained so chunk k arrives in order
            for k in range(1, NB):
                d = nc.sync.dma_start(
                    xt_sb[:, :, k * QB : (k + 1) * QB],
                    xt_r[:, :, k * QB : (k + 1) * QB],
                )
                add_dep_helper(d.ins, prev_dma.ins, sync=True, reason="xt order")
                prev_dma = d
            # weights needed only by the output projection come last
            d = nc.sync.dma_start(wo_sb[:], wo.rearrange("(s p) m -> p s m", p=P))
            add_dep_helper(d.ins, prev_dma.ins, sync=True, reason="wo after xt")
            d = nc.sync.dma_start(bob_sb[:], bob[:])
            add_dep_helper(d.ins, prev_dma.ins, sync=True, reason="bob after xt")
            nc.vector.memset(v_sb[:, :, :, DH : DH + 1], 1.0)

            # token-split A2A buffers: slot j = [128 ch, blocks, 64 tok]
            a2a_in1 = dram.tile([NW, P, SPLIT, SL], BF16, name="a2a_in1")
            a2a_out1 = dram.tile([NW, P, SPLIT, SL], BF16, name="a2a_out1")
            a2a_in2 = dram.tile([NW, P, NB - SPLIT, SL], BF16, name="a2a_in2")
            a2a_out2 = dram.tile([NW, P, NB - SPLIT, SL], BF16, name="a2a_out2")

            with (
                tc.tile_pool(name="psC", bufs=1, space="PSUM") as psC,
                tc.tile_pool(name="psT", bufs=1, space="PSUM") as psT,
                tc.tile_pool(name="sbm", bufs=2) as sbm,
            ):
                # ---- QKV step emitters (one PSUM tile each) ----

                def emit_qk_step(t0, tlen, dst, w):
                    pt = psA.tile([P, QB], F32, tag="qk", name="pt")
                    for s in range(CI):
                        nc.tensor.matmul(
                            pt[:, 0:tlen],
                            w[:, s, :],
                            xt_sb[:, s, t0 : t0 + tlen],
                            start=(s == 0),
                            stop=(s == CI - 1),
                        )
                    nc.vector.tensor_copy(dst[:, t0 : t0 + tlen], pt[:, 0:tlen])

                def emit_v_step(t0, tlen):
                    tt0 = t0 // P
                    ntt = tlen // P
                    pv = psA.tile([P, QB], F32, tag="qk", name="pv")
                    for tt in range(ntt):
                        for s in range(CI):
                            nc.tensor.matmul(
                                pv[:, tt * P : (tt + 1) * P],
                                xt_sb[:, s, (tt0 + tt) * P : (tt0 + tt + 1) * P],
                                wv_sb[:, s, :],
                                start=(s == 0),
                                stop=(s == CI - 1),
                            )
                    nc.vector.tensor_copy(
                        v_sb[:, tt0 : tt0 + ntt, :, 0:DH],
                        pv[:, 0:tlen].rearrange("p (t h d) -> p t h d", t=ntt, h=HL),
                    )

                def qkv_steps():
                    for t0, tlen in CHUNKS:
                        fi = t0 // QB
                        yield (fi, lambda t0=t0, tlen=tlen: emit_qk_step(t0, tlen, q_sb, wq_sb))
                        yield (fi, lambda t0=t0, tlen=tlen: emit_qk_step(t0, tlen, k_sb, wk_sb))
                        yield (fi, lambda t0=t0, tlen=tlen: emit_v_step(t0, tlen))

                qkv_iter = qkv_steps()
                qkv_pending = []  # one lookahead slot
                drip_tick = [0]
                ep_queue = []  # deferred per-unit epilogue thunks

                def qkv_drip(max_steps):
                    n = 0
                    while n < max_steps:
                        if qkv_pending:
                            _, thunk = qkv_pending.pop(0)
                            thunk()
                            n += 1
                            continue
                        nxt = next(qkv_iter, None)
                        if nxt is None:
                            return
                        qkv_pending.append(nxt)

                def qkv_flush(through_blk):
                    while True:
                        if qkv_pending:
                            fi, thunk = qkv_pending[0]
                            if fi > through_blk:
                                return
                            qkv_pending.pop(0)
                            thunk()
                            continue
                        nxt = next(qkv_iter, None)
                        if nxt is None:
                            return
                        qkv_pending.append(nxt)

                def ep_drip(n=1):
                    for _ in range(n):
                        if ep_queue:
                            ep_queue.pop(0)()

                def ep_flush():
                    while ep_queue:
                        ep_queue.pop(0)()

                # ---- attention emitter ----

                def emit_attn(ui, h, blk, pools, drip=False):
                    """Score groups for unit (head h, block blk).  Returns a
                    thunk emitting the unit's attn@V + epilogue — invoked
                    after the NEXT unit's score groups so the Activation
                    engine's exp stream never waits on the PE draining attn@V
                    at a unit boundary."""
                    b, qb = blk // 4, blk % 4
                    at = at_sb[:, ui % 3]
                    hp = DH * h
                    tb = b * T
                    kb = b * NKT
                    qs0 = tb + qb * QB
                    nkt = 4 * (qb + 1)  # kpos tiles up to the diagonal
                    groups = []
                    kt = 0
                    gi = 0
                    while kt < nkt:
                        pool, cap = pools[gi % len(pools)]
                        n = min(cap, nkt - kt)
                        groups.append((kt, n, pool, cap))
                        kt += n
                        gi += 1

                    for g, (kt0, n, pool, cap) in enumerate(groups):
                        sps = pool.tile([P, cap, QB], F32, tag="s", name="sps")
                        # diagonal tiles are exp'd in pairs; trim each tile's
                        # matmul only to its PAIR's q start so the paired exp
                        # never reads unwritten PSUM
                        i0 = max(0, 4 * qb - kt0)  # first diagonal index
                        for i in range(n):
                            ktg = kt0 + i
                            dq = ktg - 4 * qb
                            if dq >= 0:
                                dq_pair = (kt0 + i0 + 2 * ((i - i0) // 2)) - 4 * qb
                                lo = dq_pair * P
                            else:
                                lo = 0
                            nc.tensor.matmul(
                                sps[:, i, lo:QB],
                                k_sb[hp : hp + DH, tb + ktg * P : tb + (ktg + 1) * P],
                                q_sb[hp : hp + DH, qs0 + lo : qs0 + QB],
                                start=True,
                                stop=True,
                                tile_position=(hp, 0),
                            )
                        # exp: off-diagonal tiles full-width in one run,
                        # diagonal tiles in pairs trimmed to the pair's start
                        i = 0
                        while i < n:
                            dq0 = (kt0 + i) - 4 * qb
                            if dq0 < 0:
                                j = i
                                while j < n and (kt0 + j) - 4 * qb < 0:
                                    j += 1
                                nc.scalar.activation(
                                    at[:, kt0 + i : kt0 + j, :],
                                    sps[:, i:j, :],
                                    EXP,
                                    scale=0.125,
                                )
                                i = j
                            else:
                                j = min(i + 2, n)
                                lo = dq0 * P
                                nc.scalar.activation(
                                    at[:, kt0 + i : kt0 + j, lo:QB],
                                    sps[:, i:j, lo:QB],
                                    EXP,
                                    scale=0.125,
                                )
                                i = j
                        # causal mask on the diagonal tiles
                        for i in range(n):
                            dq = (kt0 + i) - 4 * qb
                            if dq >= 0:
                                a = at[:, kt0 + i, dq * P : (dq + 1) * P]
                                nc.vector.tensor_tensor(
                                    a, a, mskd_sb[:], mybir.AluOpType.mult
                                )
                        ep_drip(2)
                        if drip:
                            drip_tick[0] = (drip_tick[0] + 1) % 3
                            if drip_tick[0] == 0:
                                qkv_drip(1)

                    def av_ep():
                        ep_flush()
                        cps = psC.tile([P, 4, DH + 1], F32, tag="ctx", name="cps")
                        for qs in range(4):
                            qg = 4 * qb + qs
                            for kt in range(qg + 1):
                                nc.tensor.matmul(
                                    cps[:, qs, :],
                                    at[:, kt, qs * P : (qs + 1) * P],
                                    v_sb[:, kb + kt, h, :],
                                    start=(kt == 0),
                                    stop=(kt == qg),
                                )
                        ctxn = sbm.tile([P, 4, DH], BF16, tag="ctxn", name="ctxn")
                        den_sb = sbm.tile([P, 4], F32, tag="den", name="den_sb")
                        nc.vector.reciprocal(den_sb[:], cps[:, :, DH])
                        tp_box = []

                        def ep_qs(qs):
                            den = den_sb[:, qs : qs + 1]
                            denb = bass.AP(
                                tensor=den.tensor,
                                offset=den.offset,
                                ap=[list(den.ap[0]), [0, DH]],
                            )
                            nc.vector.tensor_tensor(
                                ctxn[:, qs, :],
                                cps[:, qs, 0:DH],
                                denb,
                                mybir.AluOpType.mult,
                            )
                            if not tp_box:
                                tp_box.append(
                                    psT.tile([DH, 4, P], BF16, tag="tp", name="tp")
                                )
                            nc.tensor.transpose(
                                tp_box[0][:, qs, :], ctxn[:, qs, :], iden_sb[:]
                            )

                        def ep_stage():
                            nc.vector.tensor_copy(
                                ctxT_sb[hp : hp + DH, blk, :],
                                tp_box[0][:].rearrange("d a p -> d (a p)"),
                            )
                            src = ctxT_sb[hp : hp + DH, blk, :].rearrange(
                                "d (j t) -> j d t", j=NW
                            )
                            if blk < SPLIT:
                                nc.sync.dma_start(
                                    a2a_in1[:, hp : hp + DH, blk, :], src
                                )
                            else:
                                nc.sync.dma_start(
                                    a2a_in2[:, hp : hp + DH, blk - SPLIT, :], src
                                )

                        for qs in range(4):
                            ep_queue.append(lambda qs=qs: ep_qs(qs))
                        ep_queue.append(ep_stage)

                    return av_ep

                def emit_a2a(i):
                    a_in = a2a_in1 if i == 0 else a2a_in2
                    a_out = a2a_out1 if i == 0 else a2a_out2
                    nblk = SPLIT if i == 0 else NB - SPLIT
                    c0 = 0 if i == 0 else SPLIT * SL
                    nc.gpsimd.collective_compute(
                        "AllToAll",
                        mybir.AluOpType.bypass,
                        replica_groups=[[0, 1, 2, 3, 4, 5, 6, 7]],
                        ins=[a_in.opt()],
                        outs=[a_out.opt()],
                    )
                    # land in two halves (src cores 0-3, 4-7) for pipelining
                    for g2 in range(2):
                        nc.sync.dma_start(
                            cf_sb[:, 4 * g2 : 4 * (g2 + 1), c0 : c0 + nblk * SL],
                            a_out[4 * g2 : 4 * (g2 + 1)].rearrange(
                                "s c b t -> c s (b t)"
                            ),
                        )

                # ---- attention: block-major, both heads per block ----
                units_A = [(h, blk) for blk in range(6) for h in range(HL)]
                # block 7 before 6 so the final unit (gating A2A#2) is smaller
                units_B = [(0, 7), (1, 7), (0, 6), (1, 6)]
                trig_a2a1 = (HL - 1, SPLIT - 1)
                av_prev = None
                av_prev_unit = None
                ucount = 0

                def pump(av_new, unit):
                    nonlocal av_prev, av_prev_unit, ucount
                    if av_prev is not None:
                        av_prev()
                        if av_prev_unit == trig_a2a1:
                            ep_flush()
                            emit_a2a(0)
                    av_prev = av_new
                    av_prev_unit = unit
                    ucount += 1

                with (
                    tc.tile_pool(name="psA", bufs=2, space="PSUM") as psA,
                    tc.tile_pool(name="psS", bufs=1, space="PSUM") as psS,
                    tc.tile_pool(name="psSb", bufs=1, space="PSUM") as psSb,
                ):
                    pools1 = [(psS, 2), (psSb, 2)]
                    for h, blk in units_A:
                        qkv_flush(blk)  # deps for this unit
                        av_new = emit_attn(ucount, h, blk, pools1, drip=True)
                        pump(av_new, (h, blk))
                    qkv_flush(NB)
                with (
                    tc.tile_pool(name="psS4", bufs=1, space="PSUM") as psS4,
                    tc.tile_pool(name="psS5", bufs=1, space="PSUM") as psS5,
                ):
                    pools2 = [(psS4, 3), (psS5, 3)]
                    for h, blk in units_B:
                        av_new = emit_attn(ucount, h, blk, pools2)
                        pump(av_new, (h, blk))
                    av_prev()
                    ep_flush()
                emit_a2a(1)

            # ---- phase 3: output projection, token pairs of 128 ----
            with (
                tc.tile_pool(name="psO", bufs=2, space="PSUM") as psO,
                tc.tile_pool(name="psJ", bufs=1, space="PSUM") as psJ,
                tc.tile_pool(name="osb", bufs=2) as osb,
            ):
                jt = psJ.tile([P, QB], F32, tag="junk", name="jt")

                def emit_junk(n):
                    for _ in range(n):
                        nc.tensor.matmul(
                            jt[:],
                            q_sb[0:DH, 0:P],
                            q_sb[0:DH, 0:QB],
                            start=True,
                            stop=True,
                            tile_position=(0, 0),
                            skip_group_check=True,
                        )

                def emit_pair(p):
                    # n2-outer: the first column half's bias+store overlaps
                    # the second half's matmuls
                    ot = osb.tile([P, D], F32, tag="o", name="ot")
                    for n2 in range(2):
                        po = psO.tile([P, 512], F32, tag=f"po{n2}", name="po")
                        for s in range(CI):
                            nc.tensor.matmul(
                                po[:],
                                cf_sb[:, s, p * P : (p + 1) * P],
                                wo_sb[:, s, n2 * 512 : (n2 + 1) * 512],
                                start=(s == 0),
                                stop=(s == CI - 1),
                            )
                        nc.vector.tensor_tensor(
                            ot[:, n2 * 512 : (n2 + 1) * 512],
                            po[:],
                            bob_sb[:, n2 * 512 : (n2 + 1) * 512],
                            mybir.AluOpType.add,
                        )
                        nc.sync.dma_start(
                            out[p * P : (p + 1) * P, n2 * 512 : (n2 + 1) * 512],
                            ot[:, n2 * 512 : (n2 + 1) * 512],
                        )

                emit_junk(JUNK_PRE)
                for p in range(SPLIT // 2):
                    emit_pair(p)
                emit_junk(JUNK_MID)
                for p in range(SPLIT // 2, 4):
                    emit_pair(p)
    nc.finalize()
    return nc


def _get_nc():
    if "nc" not in _CACHE:
        _CACHE["nc"] = _build()
    return _CACHE["nc"]


def kernel(x, Wq, Wk, Wv, Wo, bo, **run_kwargs):
    x = np.asarray(x, np.float32)
    Wq = np.asarray(Wq, np.float32)
    Wk = np.asarray(Wk, np.float32)
    Wv = np.asarray(Wv, np.float32)
    Wo = np.asarray(Wo, np.float32)
    bo = np.asarray(bo, np.float32)

    xt16 = np.ascontiguousarray(x.reshape(TQ, D).T).astype(ml_dtypes.bfloat16)
    wo16 = Wo.astype(ml_dtypes.bfloat16)
    bob = np.ascontiguousarray(np.broadcast_to(bo, (P, D))).astype(np.float32)
    ii = np.arange(P)[:, None]
    jj = np.arange(P)[None, :]
    mskd = (jj >= ii).astype(ml_dtypes.bfloat16)
    iden = np.eye(P, dtype=ml_dtypes.bfloat16)

    in_maps = []
    for c in range(8):
        sl = slice(P * c, P * (c + 1))
        in_maps.append(
            {
                "xt": xt16,
                "wq": np.ascontiguousarray(Wq[:, sl]).astype(ml_dtypes.bfloat16),
                "wk": np.ascontiguousarray(Wk[:, sl]).astype(ml_dtypes.bfloat16),
                "wv": np.ascontiguousarray(Wv[:, sl]).astype(ml_dtypes.bfloat16),
                "wo": wo16,
                "bob": bob,
                "mskd": mskd,
                "iden": iden,
            }
        )

    nc = _get_nc()
    res = run_bass_kernel_spmd(nc, in_maps, core_ids=list(range(8)), **run_kwargs)

    outp = np.empty((B, T, D), np.float32)
    for c in range(8):
        arr = res.results[c]["out"].reshape(NB, SL, D)
        for blk in range(NB):
            b, qb = blk // 4, blk % 4
            t0 = qb * QB + c * SL
            outp[b, t0 : t0 + SL, :] = arr[blk]
    return outp


# revision 17
# speedup vs baseline: 1.0554x; 1.0389x over previous
"""Multi-head causal attention (b=2, T=2048, d=1024, 16 heads) on 8 TRN2 cores.

Sharding: tensor-parallel over heads, 2 heads per core, both batch elements on
every core.  Per core:
  - QKV projections (contraction over d_in=1024) with x^T resident in SBUF;
    Q^T/K^T land in [channel, token] layout, V in [token, channel] layout
    augmented with a ones column (softmax denominator).
  - Attention per (head, block) unit in transposed-score layout S^T[kpos, q],
    where block = (batch, 512-token q range), processed block-major with both
    heads per block: scores (diagonal tiles trimmed to the valid q range) ->
    exp (max-free softmax, scores bounded) -> causal mask on the diagonal
    128x128 tiles -> attn@V with the exp'd scores stationary, producing
    ctx[q, ch]; denominator from the ones column -> reciprocal + broadcast
    multiply.  ctx is transposed back to ctx^T[ch, q] with PE transposes and
    staged per block.
  - Two token-split AllToAlls re-shard ctx from head-sharded to token-sharded:
    each slot carries BOTH local heads for a 64-token slice per block.  A2A#1
    covers blocks 0-5 and overlaps the tail of attention; A2A#2 covers only
    blocks 6-7 (262KB), minimizing the exposed collective latency.
  - Received ctx channels land in natural Wo row order (core-major), so the
    out-projection contracts full 128-row subtiles: token pairs (2 blocks =
    128 tokens) x 8 subtiles x 2 column halves.  Pairs 0-2 run under the
    A2A#2 window; pair 3 lands right after A2A#2.  Junk matmuls (reading
    resident q_sb) keep the PE p-state ramped across the collective windows.
  - QKV projection work is dripped between attention score groups in program
    order so the in-order PE queue always has independent matmuls while the
    Activation engine works through the exps.
Host side only shards/casts inputs and concatenates the 8 output slices.
"""

import sys

sys.path.insert(0, "/opt/trn_rl_repo")

import numpy as np
import ml_dtypes

import concourse.bass as bass
import concourse.mybir as mybir
import concourse.tile as tile
from concourse.tile import add_dep_helper
from concourse import bacc
from concourse.bass_utils import run_bass_kernel_spmd

B = 2
T = 2048
D = 1024
DH = 64
HL = 2  # heads per core
P = 128
CI = D // P  # 8 contraction subtiles
TQ = B * T  # 4096
QB = 512  # q block
NB = TQ // QB  # 8 blocks total (batch-major)
NKT = T // P  # 16 kpos tiles per batch
NW = 8  # a2a slots == cores
SPLIT = 4  # blocks 0..SPLIT-1 in A2A#1, rest in A2A#2
SL = QB // NW  # 64 tokens per (block, slot) slice
F32 = mybir.dt.float32
BF16 = mybir.dt.bfloat16
EXP = mybir.ActivationFunctionType.Exp

JUNK_PRE = 0    # 512-row junk matmuls before D1 (cf1 lands during attention)
JUNK_MID = 135  # 512-row junk matmuls between D1 and D2 (bridge to cf2)

_CACHE = {}


def _build():
    nc = bacc.Bacc("TRN2", target_bir_lowering=False, num_devices=8)
    xt = nc.dram_tensor("xt", [D, TQ], BF16, kind="ExternalInput")
    wq = nc.dram_tensor("wq", [D, P], BF16, kind="ExternalInput")
    wk = nc.dram_tensor("wk", [D, P], BF16, kind="ExternalInput")
    wv = nc.dram_tensor("wv", [D, P], BF16, kind="ExternalInput")
    wo = nc.dram_tensor("wo", [D, D], BF16, kind="ExternalInput")
    bob = nc.dram_tensor("bob", [P, D], F32, kind="ExternalInput")
    mskd = nc.dram_tensor("mskd", [P, P], BF16, kind="ExternalInput")
    iden = nc.dram_tensor("iden", [P, P], BF16, kind="ExternalInput")
    out = nc.dram_tensor("out", [QB, D], F32, kind="ExternalOutput")

    xt_r = xt.rearrange("(s p) t -> p s t", p=P)

    # x chunk schedule: small first chunks so the first QK matmuls start early
    CHUNKS = [(0, 128), (128, 384)] + [(512 * k, 512) for k in range(1, NB)]

    with tile.TileContext(nc) as tc:
        with (
            tc.tile_pool(name="const", bufs=1) as const,
            tc.tile_pool(name="dram", bufs=1, space="DRAM") as dram,
        ):
            xt_sb = const.tile([P, CI, TQ], BF16)
            wq_sb = const.tile([P, CI, P], BF16)
            wk_sb = const.tile([P, CI, P], BF16)
            wv_sb = const.tile([P, CI, P], BF16)
            wo_sb = const.tile([P, CI, D], BF16)
            bob_sb = const.tile([P, D], F32)
            mskd_sb = const.tile([P, P], BF16)
            iden_sb = const.tile([P, P], BF16)
            q_sb = const.tile([P, TQ], BF16)
            k_sb = const.tile([P, TQ], BF16)
            # V in [token, channel] layout + ones column: [kpos_tile, head, 65]
            v_sb = const.tile([P, B * NKT, HL, DH + 1], BF16)
            # exp'd scores for the current (head, block): all kpos tiles,
            # triple-buffered per unit
            at_sb = const.tile([P, 3, NKT, QB], BF16)
            # ctx^T staging: head h on partitions 64h..64h+64, per block
            ctxT_sb = const.tile([P, NB, QB], BF16)
            # re-sharded full-channel ctx for my token slices:
            # [ch-in-subtile, subtile(=src core), blk*64+t]
            cf_sb = const.tile([P, CI, QB], BF16)

            # wq + first x chunks unblock the first Q projection
            nc.sync.dma_start(wq_sb[:], wq.rearrange("(s p) m -> p s m", p=P))
            prev_dma = nc.sync.dma_start(xt_sb[:, :, 0:128], xt_r[:, :, 0:128])
            nc.sync.dma_start(wk_sb[:], wk.rearrange("(s p) m -> p s m", p=P))
            nc.sync.dma_start(wv_sb[:], wv.rearrange("(s p) m -> p s m", p=P))
            d = nc.sync.dma_start(xt_sb[:, :, 128:512], xt_r[:, :, 128:512])
            add_dep_helper(d.ins, prev_dma.ins, sync=True, reason="xt order")
            prev_dma = d
            nc.sync.dma_start(mskd_sb[:], mskd[:])
            nc.sync.dma_start(iden_sb[:], iden[:])
            # remaining x^T chunks, chained so chunk k arrives in order
            for k in range(1, NB):
                d = nc.sync.dma_start(
                    xt_sb[:, :, k * QB : (k + 1) * QB],
                    xt_r[:, :, k * QB : (k + 1) * QB],
                )
                add_dep_helper(d.ins, prev_dma.ins, sync=True, reason="xt order")
                prev_dma = d
            # weights needed only by the output projection come last
            d = nc.sync.dma_start(wo_sb[:], wo.rearrange("(s p) m -> p s m", p=P))
            add_dep_helper(d.ins, prev_dma.ins, sync=True, reason="wo after xt")
            d = nc.sync.dma_start(bob_sb[:], bob[:])
            add_dep_helper(d.ins, prev_dma.ins, sync=True, reason="bob after xt")
            nc.vector.memset(v_sb[:, :, :, DH : DH + 1], 1.0)

            # token-split A2A buffers: slot j = [128 ch, blocks, 64 tok]
            a2a_in1 = dram.tile([NW, P, SPLIT, SL], BF16, name="a2a_in1")
            a2a_out1 = dram.tile([NW, P, SPLIT, SL], BF16, name="a2a_out1")
            a2a_in2 = dram.tile([NW, P, NB - SPLIT, SL], BF16, name="a2a_in2")
            a2a_out2 = dram.tile([NW, P, NB - SPLIT, SL], BF16, name="a2a_out2")

            with (
                tc.tile_pool(name="psC", bufs=1, space="PSUM") as psC,
                tc.tile_pool(name="psT", bufs=1, space="PSUM") as psT,
                tc.tile_pool(name="sbm", bufs=2) as sbm,
            ):
                # ---- QKV step emitters (one PSUM tile each) ----

                def emit_qk_step(t0, tlen, dst, w):
                    pt = psA.tile([P, QB], F32, tag="qk", name="pt")
                    for s in range(CI):
                        nc.tensor.matmul(
                            pt[:, 0:tlen],
                            w[:, s, :],
                            xt_sb[:, s, t0 : t0 + tlen],
                            start=(s == 0),
                            stop=(s == CI - 1),
                        )
                    nc.vector.tensor_copy(dst[:, t0 : t0 + tlen], pt[:, 0:tlen])

                def emit_v_step(t0, tlen):
                    tt0 = t0 // P
                    ntt = tlen // P
                    pv = psA.tile([P, QB], F32, tag="qk", name="pv")
                    for tt in range(ntt):
                        for s in range(CI):
                            nc.tensor.matmul(
                                pv[:, tt * P : (tt + 1) * P],
                                xt_sb[:, s, (tt0 + tt) * P : (tt0 + tt + 1) * P],
                                wv_sb[:, s, :],
                                start=(s == 0),
                                stop=(s == CI - 1),
                            )
                    nc.vector.tensor_copy(
                        v_sb[:, tt0 : tt0 + ntt, :, 0:DH],
                        pv[:, 0:tlen].rearrange("p (t h d) -> p t h d", t=ntt, h=HL),
                    )

                def qkv_steps():
                    for t0, tlen in CHUNKS:
                        fi = t0 // QB
                        yield (fi, lambda t0=t0, tlen=tlen: emit_qk_step(t0, tlen, q_sb, wq_sb))
                        yield (fi, lambda t0=t0, tlen=tlen: emit_qk_step(t0, tlen, k_sb, wk_sb))
                        yield (fi, lambda t0=t0, tlen=tlen: emit_v_step(t0, tlen))

                qkv_iter = qkv_steps()
                qkv_pending = []  # one lookahead slot
                drip_tick = [0]
                ep_queue = []  # deferred per-unit epilogue thunks

                def qkv_drip(max_steps):
                    n = 0
                    while n < max_steps:
                        if qkv_pending:
                            _, thunk = qkv_pending.pop(0)
                            thunk()
                            n += 1
                            continue
                        nxt = next(qkv_iter, None)
                        if nxt is None:
                            return
                        qkv_pending.append(nxt)

                def qkv_flush(through_blk):
                    while True:
                        if qkv_pending:
                            fi, thunk = qkv_pending[0]
                            if fi > through_blk:
                                return
                            qkv_pending.pop(0)
                            thunk()
                            continue
                        nxt = next(qkv_iter, None)
                        if nxt is None:
                            return
                        qkv_pending.append(nxt)

                def ep_drip(n=1):
                    for _ in range(n):
                        if ep_queue:
                            ep_queue.pop(0)()

                def ep_flush():
                    while ep_queue:
                        ep_queue.pop(0)()

                # ---- attention emitter ----

                def emit_attn(ui, h, blk, pools, drip=False, av_cb=None,
                              inline_ep=False):
                    """Score groups for unit (head h, block blk).  Returns a
                    thunk emitting the unit's attn@V + epilogue — invoked
                    after the NEXT unit's score groups so the Activation
                    engine's exp stream never waits on the PE draining attn@V
                    at a unit boundary."""
                    b, qb = blk // 4, blk % 4
                    at = at_sb[:, ui % 3]
                    hp = DH * h
                    tb = b * T
                    kb = b * NKT
                    qs0 = tb + qb * QB
                    nkt = 4 * (qb + 1)  # kpos tiles up to the diagonal
                    groups = []
                    kt = 0
                    gi = 0
                    while kt < nkt:
                        pool, cap = pools[gi % len(pools)]
                        n = min(cap, nkt - kt)
                        groups.append((kt, n, pool, cap))
                        kt += n
                        gi += 1

                    for g, (kt0, n, pool, cap) in enumerate(groups):
                        sps = pool.tile([P, cap, QB], F32, tag="s", name="sps")
                        # diagonal tiles are exp'd in pairs; trim each tile's
                        # matmul only to its PAIR's q start so the paired exp
                        # never reads unwritten PSUM
                        i0 = max(0, 4 * qb - kt0)  # first diagonal index
                        for i in range(n):
                            ktg = kt0 + i
                            dq = ktg - 4 * qb
                            if dq >= 0:
                                dq_pair = (kt0 + i0 + 2 * ((i - i0) // 2)) - 4 * qb
                                lo = dq_pair * P
                            else:
                                lo = 0
                            nc.tensor.matmul(
                                sps[:, i, lo:QB],
                                k_sb[hp : hp + DH, tb + ktg * P : tb + (ktg + 1) * P],
                                q_sb[hp : hp + DH, qs0 + lo : qs0 + QB],
                                start=True,
                                stop=True,
                                tile_position=(hp, 0),
                            )
                        # exp: off-diagonal tiles full-width in one run,
                        # diagonal tiles in pairs trimmed to the pair's start
                        i = 0
                        while i < n:
                            dq0 = (kt0 + i) - 4 * qb
                            if dq0 < 0:
                                j = i
                                while j < n and (kt0 + j) - 4 * qb < 0:
                                    j += 1
                                nc.scalar.activation(
                                    at[:, kt0 + i : kt0 + j, :],
                                    sps[:, i:j, :],
                                    EXP,
                                    scale=0.125,
                                )
                                i = j
                            else:
                                j = min(i + 2, n)
                                lo = dq0 * P
                                nc.scalar.activation(
                                    at[:, kt0 + i : kt0 + j, lo:QB],
                                    sps[:, i:j, lo:QB],
                                    EXP,
                                    scale=0.125,
                                )
                                i = j
                        # causal mask on the diagonal tiles
                        for i in range(n):
                            dq = (kt0 + i) - 4 * qb
                            if dq >= 0:
                                a = at[:, kt0 + i, dq * P : (dq + 1) * P]
                                nc.vector.tensor_tensor(
                                    a, a, mskd_sb[:], mybir.AluOpType.mult
                                )
                        ep_drip(2)
                        if drip:
                            drip_tick[0] = (drip_tick[0] + 1) % 3
                            if drip_tick[0] == 0:
                                qkv_drip(1)
                        if g == 0 and av_cb is not None:
                            av_cb()

                    def av_ep():
                        ep_flush()
                        cps = psC.tile([P, 4, DH + 1], F32, tag="ctx", name="cps")
                        ctxn = sbm.tile([P, 4, DH], BF16, tag="ctxn", name="ctxn")
                        den_sb = sbm.tile([P, 4], F32, tag="den", name="den_sb")
                        tp_box = []

                        def ep_qs(qs):
                            den = den_sb[:, qs : qs + 1]
                            denb = bass.AP(
                                tensor=den.tensor,
                                offset=den.offset,
                                ap=[list(den.ap[0]), [0, DH]],
                            )
                            nc.vector.tensor_tensor(
                                ctxn[:, qs, :],
                                cps[:, qs, 0:DH],
                                denb,
                                mybir.AluOpType.mult,
                            )
                            if not tp_box:
                                tp_box.append(
                                    psT.tile([DH, 4, P], BF16, tag="tp", name="tp")
                                )
                            nc.tensor.transpose(
                                tp_box[0][:, qs, :], ctxn[:, qs, :], iden_sb[:]
                            )

                        def ep_stage():
                            nc.vector.tensor_copy(
                                ctxT_sb[hp : hp + DH, blk, :],
                                tp_box[0][:].rearrange("d a p -> d (a p)"),
                            )
                            src = ctxT_sb[hp : hp + DH, blk, :].rearrange(
                                "d (j t) -> j d t", j=NW
                            )
                            if blk < SPLIT:
                                nc.sync.dma_start(
                                    a2a_in1[:, hp : hp + DH, blk, :], src
                                )
                            else:
                                nc.sync.dma_start(
                                    a2a_in2[:, hp : hp + DH, blk - SPLIT, :], src
                                )

                        for qs in range(4):
                            qg = 4 * qb + qs
                            for kt in range(qg + 1):
                                nc.tensor.matmul(
                                    cps[:, qs, :],
                                    at[:, kt, qs * P : (qs + 1) * P],
                                    v_sb[:, kb + kt, h, :],
                                    start=(kt == 0),
                                    stop=(kt == qg),
                                )
                            if inline_ep:
                                # per-qs reciprocal so the epilogue chains off
                                # each AV chain as it completes
                                nc.vector.reciprocal(
                                    den_sb[:, qs : qs + 1], cps[:, qs, DH : DH + 1]
                                )
                                ep_qs(qs)
                        if inline_ep:
                            ep_stage()
                        else:
                            nc.vector.reciprocal(den_sb[:], cps[:, :, DH])
                            for qs in range(4):
                                ep_queue.append(lambda qs=qs: ep_qs(qs))
                            ep_queue.append(ep_stage)

                    return av_ep

                def emit_a2a(i):
                    a_in = a2a_in1 if i == 0 else a2a_in2
                    a_out = a2a_out1 if i == 0 else a2a_out2
                    nblk = SPLIT if i == 0 else NB - SPLIT
                    c0 = 0 if i == 0 else SPLIT * SL
                    nc.gpsimd.collective_compute(
                        "AllToAll",
                        mybir.AluOpType.bypass,
                        replica_groups=[[0, 1, 2, 3, 4, 5, 6, 7]],
                        ins=[a_in.opt()],
                        outs=[a_out.opt()],
                    )
                    # land in two halves (src cores 0-3, 4-7) for pipelining
                    for g2 in range(2):
                        nc.sync.dma_start(
                            cf_sb[:, 4 * g2 : 4 * (g2 + 1), c0 : c0 + nblk * SL],
                            a_out[4 * g2 : 4 * (g2 + 1)].rearrange(
                                "s c b t -> c s (b t)"
                            ),
                        )

                # ---- attention: block-major, both heads per block ----
                units_A = [(h, blk) for blk in range(6) for h in range(HL)]
                # block 7 before 6 so the final unit (gating A2A#2) is smaller
                units_B = [(0, 7), (1, 7), (0, 6), (1, 6)]
                trig_a2a1 = (HL - 1, SPLIT - 1)
                av_prev = None
                av_prev_unit = None
                ucount = 0

                def pump(av_new, unit):
                    nonlocal av_prev, av_prev_unit, ucount
                    if av_prev is not None:
                        av_prev()
                        if av_prev_unit == trig_a2a1:
                            ep_flush()
                            emit_a2a(0)
                    av_prev = av_new
                    av_prev_unit = unit
                    ucount += 1

                with (
                    tc.tile_pool(name="psA", bufs=2, space="PSUM") as psA,
                    tc.tile_pool(name="psS", bufs=1, space="PSUM") as psS,
                    tc.tile_pool(name="psSb", bufs=1, space="PSUM") as psSb,
                ):
                    pools1 = [(psS, 2), (psSb, 2)]
                    for h, blk in units_A:
                        qkv_flush(blk)  # deps for this unit
                        av_new = emit_attn(ucount, h, blk, pools1, drip=True)
                        pump(av_new, (h, blk))
                    qkv_flush(NB)
                with (
                    tc.tile_pool(name="psS4", bufs=1, space="PSUM") as psS4,
                    tc.tile_pool(name="psS5", bufs=1, space="PSUM") as psS5,
                ):
                    pools2 = [(psS4, 3), (psS5, 3)]
                    # in the act-backlog tail, drain the previous unit's AV
                    # right after the next unit's first score group so the
                    # final AV->stage chain is as short as possible
                    for i, (h, blk) in enumerate(units_B):
                        last = i == len(units_B) - 1
                        av_new = emit_attn(
                            ucount, h, blk, pools2,
                            av_cb=av_prev, inline_ep=last,
                        )
                        av_prev = av_new
                        ucount += 1
                    av_prev()
                    ep_flush()
                emit_a2a(1)

            # ---- phase 3: output projection, token pairs of 128 ----
            with (
                tc.tile_pool(name="psO", bufs=2, space="PSUM") as psO,
                tc.tile_pool(name="psJ", bufs=1, space="PSUM") as psJ,
                tc.tile_pool(name="osb", bufs=2) as osb,
            ):
                jt = psJ.tile([P, QB], F32, tag="junk", name="jt")

                def emit_junk(n):
                    for _ in range(n):
                        nc.tensor.matmul(
                            jt[:],
                            q_sb[0:DH, 0:P],
                            q_sb[0:DH, 0:QB],
                            start=True,
                            stop=True,
                            tile_position=(0, 0),
                            skip_group_check=True,
                        )

                def emit_pair(p):
                    # n2-outer: the first column half's bias+store overlaps
                    # the second half's matmuls
                    ot = osb.tile([P, D], F32, tag="o", name="ot")
                    for n2 in range(2):
                        po = psO.tile([P, 512], F32, tag=f"po{n2}", name="po")
                        for s in range(CI):
                            nc.tensor.matmul(
                                po[:],
                                cf_sb[:, s, p * P : (p + 1) * P],
                                wo_sb[:, s, n2 * 512 : (n2 + 1) * 512],
                                start=(s == 0),
                                stop=(s == CI - 1),
                            )
                        nc.vector.tensor_tensor(
                            ot[:, n2 * 512 : (n2 + 1) * 512],
                            po[:],
                            bob_sb[:, n2 * 512 : (n2 + 1) * 512],
                            mybir.AluOpType.add,
                        )
                        nc.sync.dma_start(
                            out[p * P : (p + 1) * P, n2 * 512 : (n2 + 1) * 512],
                            ot[:, n2 * 512 : (n2 + 1) * 512],
                        )

                emit_junk(JUNK_PRE)
                for p in range(SPLIT // 2):
                    emit_pair(p)
                emit_junk(JUNK_MID)
                for p in range(SPLIT // 2, 4):
                    emit_pair(p)
    nc.finalize()
    return nc


def _get_nc():
    if "nc" not in _CACHE:
        _CACHE["nc"] = _build()
    return _CACHE["nc"]


def kernel(x, Wq, Wk, Wv, Wo, bo, **run_kwargs):
    x = np.asarray(x, np.float32)
    Wq = np.asarray(Wq, np.float32)
    Wk = np.asarray(Wk, np.float32)
    Wv = np.asarray(Wv, np.float32)
    Wo = np.asarray(Wo, np.float32)
    bo = np.asarray(bo, np.float32)

    xt16 = np.ascontiguousarray(x.reshape(TQ, D).T).astype(ml_dtypes.bfloat16)
    wo16 = Wo.astype(ml_dtypes.bfloat16)
    bob = np.ascontiguousarray(np.broadcast_to(bo, (P, D))).astype(np.float32)
    ii = np.arange(P)[:, None]
    jj = np.arange(P)[None, :]
    mskd = (jj >= ii).astype(ml_dtypes.bfloat16)
    iden = np.eye(P, dtype=ml_dtypes.bfloat16)

    in_maps = []
    for c in range(8):
        sl = slice(P * c, P * (c + 1))
        in_maps.append(
            {
                "xt": xt16,
                "wq": np.ascontiguousarray(Wq[:, sl]).astype(ml_dtypes.bfloat16),
                "wk": np.ascontiguousarray(Wk[:, sl]).astype(ml_dtypes.bfloat16),
                "wv": np.ascontiguousarray(Wv[:, sl]).astype(ml_dtypes.bfloat16),
                "wo": wo16,
                "bob": bob,
                "mskd": mskd,
                "iden": iden,
            }
        )

    nc = _get_nc()
    res = run_bass_kernel_spmd(nc, in_maps, core_ids=list(range(8)), **run_kwargs)

    outp = np.empty((B, T, D), np.float32)
    for c in range(8):
        arr = res.results[c]["out"].reshape(NB, SL, D)
        for blk in range(NB):
            b, qb = blk // 4, blk % 4
            t0 = qb * QB + c * SL
            outp[b, t0 : t0 + SL, :] = arr[blk]
    return outp
